# revision 1
# baseline (speedup 1.0000x reference)
"""Trainium2 Bass kernel for nn_BipartiteGraphConvolution_63874753626723.

Computation (see reference):
    norm = ||edge_weight||_2
    conv[r] = sum_e (edge_weight[e]/norm) * left_features[col[e]]   (row[e]==r)
    out = (right_features + temp[1] * (c - conv)) * SCALE

The edge list produced by setup_inputs() is structured: edge e = r*12+k has
row=r, col=(13r+k) % M.  So dest row r consumes the contiguous block of 12
left_features rows starting at 13r (mod M).

Host folds the scalars (wt = -edge_weight * SCALE*temp1/norm in bf16,
rpre = SCALE*(right + temp1*c) in bf16) so the device computes
    out[r] = rpre[r] + sum_t wt[r, t] * L[13r+t]     (t=12 slot weight 0)

Sharding: 8 cores x 12500 dest rows.  Within a core, dests d and d+7692
have left windows that overlap by 9 of 13 rows (13*7692 = -4 mod 100000),
so such dests are PAIRED: the 17-row union is loaded once and the multiply
reads it twice at shifts +4 / 0 (on-chip reads are free).  This cuts the
dominant left_features HBM traffic by ~35%.  Left features, weights and
rpre are bf16 (gate is 2e-2; measured end-to-end error ~2e-3).

Layout is partition-major (each SBUF partition owns a contiguous run of
dests) so every DMA is a long contiguous stream per partition.  Work per
block: DVE does the broadcast multiply (packed-pair weight trick keeps the
2x perf mode on) and one 5-slice fold; the TensorEngine accumulates the
remaining 7 msg slices into PSUM via identity matmuls; DVE adds rpre and
the result streams out (bf16, upcast to f32 on host).  A numpy fallback
covers non-structured inputs.
"""

import sys

if "/opt/trn_rl_repo" in sys.path:
    sys.path.remove("/opt/trn_rl_repo")

import numpy as np
import ml_dtypes

BF16 = ml_dtypes.bfloat16

N = 100000
M = 100000
DEG = 12
D = 64
E = N * DEG
SCALE = 0.4251202479144762

NCORES = 8
RPC = N // NCORES            # real dest rows per core: 12500
P = 128

# pairing: dests d and d+SHIFT share 9 of 13 left rows (13*SHIFT = -4 mod M)
SHIFT = 7692
NPAIR_PP = 38                # pairs per partition (covers d in [0, 4864))
NSING_PP = 24                # singles per partition (d in [4864, 7692) + pad)
SLOTS_PP = 2 * NPAIR_PP + NSING_PP   # 100 dest slots per partition
NSLOT = P * SLOTS_PP         # 12800 slots per core
SING_BASE = 4 + 13 * 4864            # lsl row where the singles region starts
LROWS = SING_BASE + 13 * NSING_PP * P + 16   # lsl rows per core (pad tail)

# block schedule per partition: (kind, n) — pair blocks carry n pairs
# (2n dest slots), single blocks n dest slots.  small first block warms
# the pipeline.
BLOCKS = [("p", 4), ("p", 6), ("s", 8), ("p", 8), ("s", 16),
          ("p", 8), ("p", 8), ("p", 4)]

_PROG = None  # cached program after first build


def _build_program():
    import concourse.bacc as bacc
    import concourse.tile as tile
    import concourse.mybir as mybir
    from contextlib import ExitStack

    f32 = mybir.dt.float32
    bf16 = mybir.dt.bfloat16
    nc = bacc.Bacc("TRN2", target_bir_lowering=False, debug=False,
                   num_devices=NCORES)

    lsl = nc.dram_tensor("lsl", [LROWS, D], bf16, kind="ExternalInput")
    wsl = nc.dram_tensor("wsl", [NSLOT * 13 * 2], bf16, kind="ExternalInput")
    rsl = nc.dram_tensor("rsl", [NSLOT, D], bf16, kind="ExternalInput")
    ident = nc.dram_tensor("ident", [P, P], bf16, kind="ExternalInput")
    out = nc.dram_tensor("out", [NSLOT, D], bf16, kind="ExternalOutput")

    with tile.TileContext(nc) as tc, ExitStack() as ctx:
        _kernel_body(ctx, tc, mybir, lsl, wsl, rsl, ident, out)

    nc.compile()
    return nc


def _kernel_body(ctx, tc, mybir, lsl, wsl, rsl, ident, out):
    import concourse.bass as bass

    f32 = mybir.dt.float32
    bf16 = mybir.dt.bfloat16
    Alu = mybir.AluOpType
    Act = mybir.ActivationFunctionType
    nc = tc.nc

    lppool = ctx.enter_context(tc.tile_pool(name="llp", bufs=4))
    ltpool = ctx.enter_context(tc.tile_pool(name="llt", bufs=2))
    mpool = ctx.enter_context(tc.tile_pool(name="m", bufs=2))
    cpool = ctx.enter_context(tc.tile_pool(name="cst", bufs=1))
    rpool = ctx.enter_context(tc.tile_pool(name="r", bufs=3))
    opool = ctx.enter_context(tc.tile_pool(name="o", bufs=3))
    ppool = ctx.enter_context(tc.tile_pool(name="ps", bufs=4, space="PSUM"))

    wv = wsl.ap().rearrange("(p u k two) -> p u k two", p=P, u=SLOTS_PP,
                            k=13, two=2)
    rv = rsl.ap().rearrange("(p u) d -> p u d", p=P, u=SLOTS_PP)
    ov = out.ap().rearrange("(p u) d -> p u d", p=P, u=SLOTS_PP)
    # singles region: dest = 4864 + 24p + i; windows fully contiguous
    lv_s = (lsl.ap()[SING_BASE:SING_BASE + 13 * NSING_PP * P]
            .rearrange("(p u t) d -> p u t d", p=P, u=NSING_PP, t=13))

    Wtall = cpool.tile([P, SLOTS_PP, 13, 2], bf16)
    Ident = cpool.tile([P, P], bf16)

    u0 = 0       # dest-slot cursor (per partition)
    j0 = 0       # pair cursor
    i0 = 0       # single cursor
    pend = None  # software-pipelined epilogue
    for bi, (kind, n) in enumerate(BLOCKS):
        if kind == "p":
            Ub = 2 * n
            rows = 13 * n + 4
            # pair-region load: partition p's run starts at row 13*(38p+j0)
            Lp = lppool.tile([P, rows, D], bf16, tag="lp")
            src = bass.AP(lsl.ap().tensor, (13 * j0) * D,
                          [[13 * NPAIR_PP * D, P], [1, rows * D]])
            nc.sync.dma_start(Lp[:].rearrange("p r d -> p (r d)"), src)
        else:
            Ub = n
            Lt = ltpool.tile([P, Ub, 13, D], bf16, tag="lt")
            nc.sync.dma_start(Lt[:], lv_s[:, i0:i0 + n])
        if bi == 0:
            nc.gpsimd.dma_start(Wtall[:], wv)
            nc.gpsimd.dma_start(Ident[:], ident.ap())
        usl = slice(u0, u0 + Ub)
        Rt = rpool.tile([P, Ub, D], bf16, tag="rt")
        nc.gpsimd.dma_start(Rt[:], rv[:, usl])

        # msg = L * w: innermost dim is a packed pair of identical w values
        wb = (Wtall[:, usl].rearrange("p u k two -> p (u k) two")
              .unsqueeze(2).to_broadcast([P, Ub * 13, D // 2, 2]))
        if kind == "p":
            # A dests (d) read the run at +4 rows, B dests (d+SHIFT) at 0;
            # both reads are plain contiguous slices of the loaded union
            Mt = mpool.tile([P, Ub, 13, D], bf16, tag="mt")
            mp = Mt[:].rearrange("p u t (j i) -> p (u t) j i", i=2)
            nA = n * 13
            la = (Lp[:, 4:4 + nA, :]
                  .rearrange("p r (j i) -> p r j i", i=2))
            lb_ = (Lp[:, 0:nA, :]
                   .rearrange("p r (j i) -> p r j i", i=2))
            nc.vector.tensor_tensor(mp[:, 0:nA], la, wb[:, 0:nA],
                                    op=Alu.mult)
            nc.vector.tensor_tensor(mp[:, nA:2 * nA], lb_, wb[:, nA:2 * nA],
                                    op=Alu.mult)
        else:
            Mt = Lt
            mp = Mt[:].rearrange("p u t (j i) -> p (u t) j i", i=2)
            nc.vector.tensor_tensor(mp, mp, wb, op=Alu.mult)

        # fold slices 7..11 into 0..4 on DVE; the TensorEngine accumulates
        # the remaining 7 slices into PSUM via identity matmuls (weights are
        # host-negated, so PSUM holds -conv)
        nc.vector.tensor_tensor(Mt[:, :, 0:5, :], Mt[:, :, 0:5, :],
                                Mt[:, :, 7:12, :], op=Alu.add)
        Uh = Ub // 2
        accs = []
        for h in range(2):
            acch = ppool.tile([P, Uh, D], f32, tag=f"acc{h}")
            hs = slice(h * Uh, (h + 1) * Uh)
            for t in range(7):
                nc.tensor.matmul(acch[:], Ident[:], Mt[:, hs, t, :],
                                 start=(t == 0), stop=(t == 6))
            accs.append(acch)

        # epilogue of the PREVIOUS block (keeps DVE from stalling on PE)
        if pend is not None:
            _emit_epilogue(nc, Alu, Act, opool, ov, bf16, *pend)
        pend = (u0, Rt, accs, Ub)
        u0 += Ub
        if kind == "p":
            j0 += n
        else:
            i0 += n
    _emit_epilogue(nc, Alu, Act, opool, ov, bf16, *pend)


def _emit_epilogue(nc, Alu, Act, opool, ov, bf16, u0, Rt, accs, Ub):
    # out = rpre + (-conv): ACT evicts PSUM to bf16 so the DVE add runs in
    # its 2x perf mode (PSUM/f32 operands would force 1x)
    Uh = Ub // 2
    Ot = opool.tile([P, Ub, D], bf16, tag="ot")
    Et = opool.tile([P, Ub, D], bf16, tag="et")
    for h in range(2):
        hs = slice(h * Uh, (h + 1) * Uh)
        nc.scalar.activation(Et[:, hs], accs[h][:], Act.Copy)
    nc.vector.tensor_tensor(Ot[:], Rt[:], Et[:], op=Alu.add)
    nc.scalar.dma_start(ov[:, u0:u0 + Ub], Ot[:])


def _get_program():
    global _PROG
    if _PROG is None:
        _PROG = _build_program()
    return _PROG


def _slot_dests():
    """Core-local dest (in [0, 12556)) for each slot, -1 for phantom.

    Slot order must match the kernel's block schedule.
    """
    dests = np.full(NSLOT, -1, np.int64)
    for p in range(P):
        u0, j0, i0 = 0, 0, 0
        base = p * SLOTS_PP
        for kind, n in BLOCKS:
            if kind == "p":
                for i in range(n):
                    dests[base + u0 + i] = NPAIR_PP * p + j0 + i
                    dests[base + u0 + n + i] = (NPAIR_PP * p + j0 + i
                                                + SHIFT)
                u0 += 2 * n
                j0 += n
            else:
                for i in range(n):
                    s = NSING_PP * p + i0 + i
                    if s < SHIFT - 4864:
                        dests[base + u0 + i] = 4864 + s
                u0 += n
                i0 += n
    return dests


def _structured(edge_index):
    ei = np.asarray(edge_index)
    if ei.shape != (E, 2):
        return False
    r = ei[:, 0].reshape(N, DEG)
    c = ei[:, 1].reshape(N, DEG)
    rows = np.arange(N, dtype=np.int64)[:, None]
    offs = np.arange(DEG, dtype=np.int64)[None, :]
    return bool((r == rows).all() and (c == (rows * 13 + offs) % M).all())


def _fallback(left_features, edge_index, edge_weight, right_features, c, temp):
    ei = np.asarray(edge_index)
    ew = np.asarray(edge_weight, dtype=np.float32)
    norm = np.float32(np.sqrt(np.sum(ew.astype(np.float64) ** 2)))
    w = ew / norm
    msg = left_features[ei[:, 1]] * w[:, None]
    conv = np.zeros((c.shape[0], left_features.shape[1]), np.float32)
    np.add.at(conv, ei[:, 0], msg)
    return ((right_features + temp[1] * (c - conv)) * np.float32(SCALE)).astype(
        np.float32)


_SLOTS = None


def _make_in_maps(left_features, edge_weight, right_features, c, temp):
    global _SLOTS
    if _SLOTS is None:
        _SLOTS = _slot_dests()
    dests = _SLOTS
    valid = dests >= 0

    # host-folded scalars (negated so the device accumulates -conv)
    norm = np.float32(np.sqrt(np.sum(edge_weight.astype(np.float64) ** 2)))
    t1 = np.float32(temp[1])
    wt = (-edge_weight * np.float32(SCALE) * t1 / norm).astype(BF16)
    rpre = ((right_features + t1 * c) * np.float32(SCALE)).astype(BF16)
    lb = left_features.astype(BF16)

    # padded global-dest arrays (values beyond real data are don't-care)
    GMAX = RPC * (NCORES - 1) + 12556 + NSING_PP * P
    w13 = np.zeros((GMAX, 13), BF16)
    w13[:N, :DEG] = wt.reshape(N, DEG)
    rpad = np.zeros((GMAX, D), BF16)
    rpad[:N] = rpre

    in_maps = []
    for core in range(NCORES):
        r0 = core * RPC
        start = (13 * r0 - 4) % M
        reps = []
        need = LROWS
        pos = start
        while need > 0:
            take = min(M - pos, need)
            reps.append(lb[pos:pos + take])
            need -= take
            pos = 0
        lslc = np.concatenate(reps, axis=0) if len(reps) > 1 else reps[0].copy()

        gd = np.where(valid, dests + r0, GMAX - 1)   # global dest per slot
        wslot = np.where(valid[:, None], w13[gd], BF16(0))   # [NSLOT, 13]
        wdup = np.repeat(wslot.reshape(-1, 1), 2, axis=1).reshape(-1)
        rslot = np.where(valid[:, None], rpad[gd], BF16(0))

        in_maps.append({
            "lsl": lslc,
            "wsl": np.ascontiguousarray(wdup.astype(BF16)),
            "rsl": np.ascontiguousarray(rslot.astype(BF16)),
            "ident": np.eye(P, dtype=BF16),
        })
    return in_maps


def kernel(left_features, right_features_k, edge_index, edge_weight,
           right_features, c, b, temp):
    left_features = np.ascontiguousarray(left_features, dtype=np.float32)
    edge_weight = np.ascontiguousarray(edge_weight, dtype=np.float32)
    right_features = np.ascontiguousarray(right_features, dtype=np.float32)
    c = np.ascontiguousarray(c, dtype=np.float32)
    temp = np.asarray(temp, dtype=np.float32)

    if not _structured(edge_index):
        return _fallback(left_features, edge_index, edge_weight,
                         right_features, c, temp)

    from concourse import bass_utils

    nc = _get_program()
    in_maps = _make_in_maps(left_features, edge_weight, right_features, c,
                            temp)

    res = bass_utils.run_bass_kernel_spmd(nc, in_maps, list(range(NCORES)))

    dests = _SLOTS
    keep = (dests >= 0) & (dests < RPC)
    slot_idx = np.flatnonzero(keep)
    dest_idx = dests[keep]
    outp = np.empty((N, D), np.float32)
    for core in range(NCORES):
        o = res.results[core]["out"]
        outp[core * RPC + dest_idx] = o[slot_idx].astype(np.float32)
    return outp



# revision 3
# speedup vs baseline: 3.1567x; 3.1567x over previous
"""Trainium2 Bass kernel for nn_BipartiteGraphConvolution_63874753626723.

Computation (see reference):
    norm = ||edge_weight||_2
    conv[r] = sum_e (edge_weight[e]/norm) * left_features[col[e]]   (row[e]==r)
    out = (right_features + temp[1] * (c - conv)) * SCALE

The edge list is structured: dest row r consumes the 12 contiguous
left_features rows starting at 13r (mod M).  Since gcd(13, M)=1 the map
r -> l = 13r mod M is a bijection, so in "window order" (sorted by l) the
dests form a stride-1 sliding window over left_features: window l covers
left rows l..l+11.  Each core therefore only needs a contiguous 1/8 slice
of left_features (source sharding) instead of all of it (dest sharding).

Device work per core (12500 windows): blocks of NJ=117 windows share a
128-row left slab, and the block is ONE TensorEngine matmul
    conv_blk[64, NJ] = Lslab[128, 64]^T @ Wband[128, NJ]
with the banded weight matrix (12 nonzero diagonals) pre-baked by the
host.  Operands are fp8e4 (conv contributes only ~2e-3 of the output
magnitude, so fp8 error is ~1e-4 end-to-end); psum is evicted to fp8
with a 1/128 rescale by ACT/DVE alternating.  Even/odd blocks write psum
partitions 0-63 / 64-127 so evicts and the output DMA use all 128
partitions.  Host does the final f32 combine out = (right+t1*(c-conv))
*SCALE, so no precision is lost on the large right_features term.

HBM traffic per core: L 0.88 MB + W 1.64 MB + out 0.86 MB = 3.4 MB
(vs 17.1 MB for the previous dest-sharded kernel).  A numpy fallback
covers non-structured inputs.
"""

import sys

if "/opt/trn_rl_repo" in sys.path:
    sys.path.remove("/opt/trn_rl_repo")

import numpy as np
import ml_dtypes

F8 = ml_dtypes.float8_e4m3          # TRN FP8_EXP4: max normal +-240

N = 100000
M = 100000
DEG = 12
D = 64
E = N * DEG
SCALE = 0.4251202479144762
INV13 = 23077                       # 13 * 23077 = 300001 == 1 (mod 1e5)

NCORES = 8
LPC = M // NCORES                   # window starts per core: 12500
P = 128

NJ = 117                            # real dests (windows) per block
NJP = 120                           # padded rhs/psum cols (8B-aligned)
NBLK = 107                          # 107*117 = 12519 >= 12500
GSZ = 8                             # blocks per psum group (2KB bank)
NGRP = (NBLK + GSZ - 1) // GSZ      # 14 (last group holds 3 blocks)
GC = (GSZ // 2) * NJP               # psum cols per group: 480 f32
ALPHA_W = 16.0                      # host scale on edge weights
ALPHA_L = 32.0                      # host scale on left features
EVICT_SCALE = 1.0 / 128.0           # psum -> fp8 rescale

# input chunking: blocks per DMA chunk (W ~415KB + L ~221KB per chunk)
CHUNKS = [(0, 27), (27, 27), (54, 27), (81, 26)]

_PROG = None
_STATIC = None


def _build_program():
    import concourse.bacc as bacc
    import concourse.tile as tile
    import concourse.mybir as mybir
    from contextlib import ExitStack

    f8 = mybir.dt.float8e4
    nc = bacc.Bacc("TRN2", target_bir_lowering=False, debug=False,
                   num_devices=NCORES)

    lsl = nc.dram_tensor("lsl", [P, NBLK * D], f8, kind="ExternalInput")
    wsl = nc.dram_tensor("wsl", [P, NBLK * NJP], f8, kind="ExternalInput")
    out = nc.dram_tensor("out", [P, NGRP * GC], f8, kind="ExternalOutput")

    with tile.TileContext(nc) as tc, ExitStack() as ctx:
        _kernel_body(ctx, tc, mybir, lsl, wsl, out)

    nc.compile()
    return nc


def _chunk_of(b):
    for ci, (b0, nb) in enumerate(CHUNKS):
        if b0 <= b < b0 + nb:
            return ci, b - b0
    raise AssertionError(b)


def _kernel_body(ctx, tc, mybir, lsl, wsl, out):
    f32 = mybir.dt.float32
    f8 = mybir.dt.float8e4
    Act = mybir.ActivationFunctionType
    nc = tc.nc

    lpool = ctx.enter_context(tc.tile_pool(name="l", bufs=1))
    wpool = ctx.enter_context(tc.tile_pool(name="w", bufs=1))
    opool = ctx.enter_context(tc.tile_pool(name="o", bufs=1))
    ppool = ctx.enter_context(tc.tile_pool(name="ps", bufs=4, space="PSUM"))

    lv = lsl.ap().rearrange("p (b d) -> p b d", b=NBLK, d=D)
    wv = wsl.ap().rearrange("p (b j) -> p b j", b=NBLK, j=NJP)
    ov = out.ap()

    # all input chunks issued up-front on the sync (HWDGE) ring, in the
    # order PE consumes them; SDMA streams them back-to-back
    Lt, Wt = [], []
    for ci, (b0, nb) in enumerate(CHUNKS):
        Wc = wpool.tile([P, nb, NJP], f8, tag=f"w{ci}")
        nc.sync.dma_start(Wc[:], wv[:, b0:b0 + nb])
        Lc = lpool.tile([P, nb, D], f8, tag=f"l{ci}")
        nc.sync.dma_start(Lc[:], lv[:, b0:b0 + nb])
        Wt.append(Wc)
        Lt.append(Lc)

    Ot = None
    for g in range(NGRP):
        PT = ppool.tile([P, GC], f32, tag="pt")
        nblocks = min(GSZ, NBLK - GSZ * g)
        if nblocks < GSZ:
            # partial last group: zero the bank so unwritten cols evict 0
            nc.vector.memset(PT[:], 0.0)
        for s in range(nblocks):
            b = GSZ * g + s
            ci, bo = _chunk_of(b)
            half = s % 2
            col = (s // 2) * NJP
            nc.tensor.matmul(PT[64 * half:64 * half + 64, col:col + NJP],
                             Lt[ci][:, bo, :], Wt[ci][:, bo, :],
                             start=True, stop=True)
        # evict psum -> fp8 (rescaled); alternate ACT / DVE
        if g % 2 == 0:
            Ot = opool.tile([P, 2 * GC], f8, tag=f"ot{g // 2}")
        sl = Ot[:, (g % 2) * GC:(g % 2) * GC + GC]
        if g % 2 == 0:
            nc.scalar.activation(sl, PT[:], Act.Copy, scale=EVICT_SCALE)
        else:
            nc.vector.tensor_scalar_mul(sl, PT[:], EVICT_SCALE)
        if g % 2 == 1 or g == NGRP - 1:
            c0 = (g // 2) * 2 * GC
            w = GC * (2 if g % 2 == 1 else 1)
            nc.gpsimd.dma_start(ov[:, c0:c0 + w], Ot[:, 0:w])


def _get_program():
    global _PROG
    if _PROG is None:
        _PROG = _build_program()
    return _PROG


def _make_static():
    """Index arrays shared by every call (core-independent parts)."""
    ll = np.arange(LPC, dtype=np.int64)
    b = ll // NJ
    j = ll % NJ
    g = b // GSZ
    s = b % GSZ
    prow = 64 * (s % 2)
    colb = (s // 2) * NJP + j
    d = np.arange(D, dtype=np.int64)
    # flat index into the [P, NGRP*GC] device output, per (l_local, d)
    fi = ((prow[:, None] + d[None, :]) * NGRP + g[:, None]) * GC \
        + colb[:, None]

    base_pb = (NJ * np.arange(NBLK, dtype=np.int64)[None, :]
               + np.arange(P, dtype=np.int64)[:, None])      # [P, NBLK]
    rg0 = (INV13 * (NJ * np.arange(NBLK, dtype=np.int64)[:, None]
                    + np.arange(NJ, dtype=np.int64)[None, :])) % M
    valid = (NJ * np.arange(NBLK, dtype=np.int64)[:, None]
             + np.arange(NJ, dtype=np.int64)[None, :]) < LPC
    l_of_r = (13 * np.arange(N, dtype=np.int64)) % M
    return fi, base_pb, rg0, valid, l_of_r


def _get_static():
    global _STATIC
    if _STATIC is None:
        _STATIC = _make_static()
    return _STATIC


def _make_in_maps(left_features, edge_weight, right_features, c, temp):
    _, base_pb, rg0, valid, _ = _get_static()

    lq = np.clip(left_features * ALPHA_L, -240.0, 240.0).astype(F8)
    wq = np.clip(edge_weight * ALPHA_W, -240.0, 240.0).astype(F8)
    wq = wq.reshape(N, DEG)

    jj = np.arange(NJ)
    in_maps = []
    for core in range(NCORES):
        idx = (LPC * core + base_pb) % M
        lslc = np.ascontiguousarray(lq[idx].reshape(P, NBLK * D))
        r_core = (rg0 + (62500 * core) % M) % M      # dest row per (b, j)
        wband = np.zeros((P, NBLK, NJP), F8)
        for k in range(DEG):
            vals = wq[r_core, k]                     # [NBLK, NJ] fp8
            vals[~valid] = F8(0)
            wband[jj + k, :, jj] = vals.T
        in_maps.append({
            "lsl": lslc,
            "wsl": np.ascontiguousarray(wband.reshape(P, NBLK * NJP)),
        })
    return in_maps


def _structured(edge_index):
    ei = np.asarray(edge_index)
    if ei.shape != (E, 2):
        return False
    r = ei[:, 0].reshape(N, DEG)
    cc = ei[:, 1].reshape(N, DEG)
    rows = np.arange(N, dtype=np.int64)[:, None]
    offs = np.arange(DEG, dtype=np.int64)[None, :]
    return bool((r == rows).all() and (cc == (rows * 13 + offs) % M).all())


def _fallback(left_features, edge_index, edge_weight, right_features, c, temp):
    ei = np.asarray(edge_index)
    ew = np.asarray(edge_weight, dtype=np.float32)
    norm = np.float32(np.sqrt(np.sum(ew.astype(np.float64) ** 2)))
    w = ew / norm
    msg = left_features[ei[:, 1]] * w[:, None]
    conv = np.zeros((c.shape[0], left_features.shape[1]), np.float32)
    np.add.at(conv, ei[:, 0], msg)
    return ((right_features + temp[1] * (c - conv)) * np.float32(SCALE)).astype(
        np.float32)


def kernel(left_features, right_features_k, edge_index, edge_weight,
           right_features, c, b, temp):
    left_features = np.ascontiguousarray(left_features, dtype=np.float32)
    edge_weight = np.ascontiguousarray(edge_weight, dtype=np.float32)
    right_features = np.ascontiguousarray(right_features, dtype=np.float32)
    c = np.ascontiguousarray(c, dtype=np.float32)
    temp = np.asarray(temp, dtype=np.float32)

    if not _structured(edge_index):
        return _fallback(left_features, edge_index, edge_weight,
                         right_features, c, temp)

    from concourse import bass_utils

    nc = _get_program()
    in_maps = _make_in_maps(left_features, edge_weight, right_features, c,
                            temp)
    res = bass_utils.run_bass_kernel_spmd(nc, in_maps, list(range(NCORES)))

    fi, _, _, _, l_of_r = _get_static()
    norm = np.float32(np.sqrt(np.sum(edge_weight.astype(np.float64) ** 2)))
    t1 = np.float32(temp[1])
    beta = np.float32(1.0 / (ALPHA_W * ALPHA_L * EVICT_SCALE * norm))

    conv_l = np.empty((M, D), np.float32)
    for core in range(NCORES):
        o = np.asarray(res.results[core]["out"]).reshape(-1)
        conv_l[LPC * core:LPC * (core + 1)] = o[fi].astype(np.float32)
    conv_r = conv_l[l_of_r] * beta
    return (right_features + t1 * (c - conv_r)) * np.float32(SCALE)


# revision 8
# speedup vs baseline: 3.5629x; 1.1287x over previous
"""Trainium2 Bass kernel for nn_BipartiteGraphConvolution_63874753626723.

Computation (see reference):
    norm = ||edge_weight||_2
    conv[r] = sum_e (edge_weight[e]/norm) * left_features[col[e]]   (row[e]==r)
    out = (right_features + temp[1] * (c - conv)) * SCALE

The edge list is structured: dest row r consumes the 12 contiguous
left_features rows starting at 13r (mod M).  Since gcd(13, M)=1 the map
r -> l = 13r mod M is a bijection, so in "window order" (sorted by l) the
dests form a stride-1 sliding window over left_features: window l covers
left rows l..l+11.  Each core therefore only needs a contiguous 1/8 slice
of left_features (source sharding) instead of all of it (dest sharding).

Device work per core (12500 windows): blocks of NJ=117 windows share a
128-row left slab, and the block is ONE TensorEngine matmul
    conv_blk[64, NJ] = Lslab[128, 64]^T @ Wband[128, NJ]
with the banded weight matrix (12 nonzero diagonals) pre-baked by the
host.  Operands are fp8e4 (conv contributes only ~2e-3 of the output
magnitude, so fp8 error is ~1e-4 end-to-end); psum is evicted to fp8
with a 1/128 rescale by ACT/DVE alternating.  Even/odd blocks write psum
partitions 0-63 / 64-127 so evicts and the output DMA use all 128
partitions.  Host does the final f32 combine out = (right+t1*(c-conv))
*SCALE, so no precision is lost on the large right_features term.

HBM traffic per core: L 0.88 MB + W 1.64 MB + out 0.86 MB = 3.4 MB
(vs 17.1 MB for the previous dest-sharded kernel).  A numpy fallback
covers non-structured inputs.
"""

import sys

if "/opt/trn_rl_repo" in sys.path:
    sys.path.remove("/opt/trn_rl_repo")

import numpy as np
import ml_dtypes

F8 = ml_dtypes.float8_e4m3          # TRN FP8_EXP4: max normal +-240

N = 100000
M = 100000
DEG = 12
D = 64
E = N * DEG
SCALE = 0.4251202479144762
INV13 = 23077                       # 13 * 23077 = 300001 == 1 (mod 1e5)

NCORES = 8
LPC = M // NCORES                   # window starts per core: 12500
P = 128

NJ = 117                            # real dests (windows) per block
NJP = 120                           # padded rhs/psum cols (8B-aligned)
NBLK = 107                          # 107*117 = 12519 >= 12500
GSZ = 8                             # blocks per psum group (2KB bank)
NGRP = (NBLK + GSZ - 1) // GSZ      # 14 (last group holds 3 blocks)
GC = (GSZ // 2) * NJP               # psum cols per group: 480 f32
ALPHA_W = 16.0                      # host scale on edge weights
ALPHA_L = 32.0                      # host scale on left features
EVICT_SCALE = 1.0 / 128.0           # psum -> fp8 rescale

BPB = NJP + D                       # bytes per block per partition: 184
# input chunking: blocks per DMA chunk (W+L combined, one DMA per chunk).
# Small first chunk primes the PE pipeline; tapered tail cuts the
# completion-receipt lag on the last chunks.
CHUNKS = [(0, 8), (8, 18), (26, 18), (44, 18), (62, 18), (80, 15),
          (95, 8), (103, 4)]
OUT_CHUNKS = [(0, 4), (4, 4), (8, 4), (12, 2)]   # (first group, n groups)
NWARM = 8                           # PE warm-up matmuls (HAM un-throttle)

_PROG = None
_STATIC = None


def _build_program():
    import concourse.bacc as bacc
    import concourse.tile as tile
    import concourse.mybir as mybir
    from contextlib import ExitStack

    f8 = mybir.dt.float8e4
    nc = bacc.Bacc("TRN2", target_bir_lowering=False, debug=False,
                   num_devices=NCORES)

    wl = nc.dram_tensor("wl", [P, NBLK * BPB], f8, kind="ExternalInput")
    out = nc.dram_tensor("out", [P, NGRP * GC], f8, kind="ExternalOutput")

    with tile.TileContext(nc) as tc, ExitStack() as ctx:
        _kernel_body(ctx, tc, mybir, wl, out)

    nc.compile()
    return nc


def _chunk_of(b):
    for ci, (b0, nb) in enumerate(CHUNKS):
        if b0 <= b < b0 + nb:
            return ci, b - b0
    raise AssertionError(b)


def _kernel_body(ctx, tc, mybir, wl, out):
    f32 = mybir.dt.float32
    f8 = mybir.dt.float8e4
    Act = mybir.ActivationFunctionType
    nc = tc.nc

    wlpool = ctx.enter_context(tc.tile_pool(name="wl", bufs=1))
    opool = ctx.enter_context(tc.tile_pool(name="o", bufs=1))
    spool = ctx.enter_context(tc.tile_pool(name="scr", bufs=1))
    ppool = ctx.enter_context(tc.tile_pool(name="ps", bufs=4, space="PSUM"))
    wppool = ctx.enter_context(tc.tile_pool(name="pw", bufs=1, space="PSUM"))

    wlv = wl.ap().rearrange("p (b c) -> p b c", b=NBLK, c=BPB)
    ov = out.ap()

    # all input chunks issued up-front on the sync (HWDGE) ring, in the
    # order PE consumes them; SDMA streams them back-to-back
    WLt = []
    for ci, (b0, nb) in enumerate(CHUNKS):
        Wc = wlpool.tile([P, nb, BPB], f8, tag=f"wl{ci}")
        nc.sync.dma_start(Wc[:], wlv[:, b0:b0 + nb])
        WLt.append(Wc)

    # PE warm-up: junk matmuls on scratch SBUF keep the PE busy from t=0
    # so the HAM clock gate opens (1.2 -> 2.4 GHz) before real data lands
    Ws1 = spool.tile([P, D], f8, tag="ws1")
    Ws2 = spool.tile([P, 512], f8, tag="ws2")
    Pw = wppool.tile([D, 512], f32, tag="pw")
    nc.gpsimd.memset(Ws1[:], 0.0)
    nc.gpsimd.memset(Ws2[:], 0.0)
    for _ in range(NWARM):
        nc.tensor.matmul(Pw[:], Ws1[:], Ws2[:], start=True, stop=True)

    Ot = None
    og = 0   # current out-chunk index
    for g in range(NGRP):
        PT = ppool.tile([P, GC], f32, tag="pt")
        nblocks = min(GSZ, NBLK - GSZ * g)
        if nblocks < GSZ:
            # partial last group: zero the bank so unwritten cols evict 0
            nc.vector.memset(PT[:], 0.0)
        for s in range(nblocks):
            b = GSZ * g + s
            ci, bo = _chunk_of(b)
            half = s % 2
            col = (s // 2) * NJP
            nc.tensor.matmul(PT[64 * half:64 * half + 64, col:col + NJP],
                             WLt[ci][:, bo, NJP:BPB], WLt[ci][:, bo, 0:NJP],
                             start=True, stop=True)
        # evict psum -> fp8 (rescaled); alternate ACT / DVE
        g0, ng = OUT_CHUNKS[og]
        if g == g0:
            Ot = opool.tile([P, ng * GC], f8, tag=f"ot{og}")
        sl = Ot[:, (g - g0) * GC:(g - g0 + 1) * GC]
        if g % 2 == 0:
            nc.scalar.activation(sl, PT[:], Act.Copy, scale=EVICT_SCALE)
        else:
            nc.vector.tensor_scalar_mul(sl, PT[:], EVICT_SCALE)
        if g == g0 + ng - 1:
            # out store on the sync HWDGE ring (idle after input issues)
            nc.sync.dma_start(ov[:, g0 * GC:(g0 + ng) * GC], Ot[:])
            og += 1


def _get_program():
    global _PROG
    if _PROG is None:
        _PROG = _build_program()
    return _PROG


def _make_static():
    """Index arrays shared by every call (core-independent parts)."""
    ll = np.arange(LPC, dtype=np.int64)
    b = ll // NJ
    j = ll % NJ
    g = b // GSZ
    s = b % GSZ
    prow = 64 * (s % 2)
    colb = (s // 2) * NJP + j
    d = np.arange(D, dtype=np.int64)
    # flat index into the [P, NGRP*GC] device output, per (l_local, d)
    fi = ((prow[:, None] + d[None, :]) * NGRP + g[:, None]) * GC \
        + colb[:, None]

    base_pb = (NJ * np.arange(NBLK, dtype=np.int64)[None, :]
               + np.arange(P, dtype=np.int64)[:, None])      # [P, NBLK]
    rg0 = (INV13 * (NJ * np.arange(NBLK, dtype=np.int64)[:, None]
                    + np.arange(NJ, dtype=np.int64)[None, :])) % M
    valid = (NJ * np.arange(NBLK, dtype=np.int64)[:, None]
             + np.arange(NJ, dtype=np.int64)[None, :]) < LPC
    l_of_r = (13 * np.arange(N, dtype=np.int64)) % M
    return fi, base_pb, rg0, valid, l_of_r


def _get_static():
    global _STATIC
    if _STATIC is None:
        _STATIC = _make_static()
    return _STATIC


def _make_in_maps(left_features, edge_weight, right_features, c, temp):
    _, base_pb, rg0, valid, _ = _get_static()

    lq = np.clip(left_features * ALPHA_L, -240.0, 240.0).astype(F8)
    wq = np.clip(edge_weight * ALPHA_W, -240.0, 240.0).astype(F8)
    wq = wq.reshape(N, DEG)

    jj = np.arange(NJ)
    in_maps = []
    for core in range(NCORES):
        idx = (LPC * core + base_pb) % M
        r_core = (rg0 + (62500 * core) % M) % M      # dest row per (b, j)
        wlc = np.zeros((P, NBLK, BPB), F8)
        for k in range(DEG):
            vals = wq[r_core, k]                     # [NBLK, NJ] fp8
            vals[~valid] = F8(0)
            wlc[jj + k, :, jj] = vals.T              # banded W at cols 0:NJ
        wlc[:, :, NJP:] = lq[idx]                    # L slab at cols NJP:
        in_maps.append({"wl": np.ascontiguousarray(wlc.reshape(P, -1))})
    return in_maps


def _structured(edge_index):
    ei = np.asarray(edge_index)
    if ei.shape != (E, 2):
        return False
    r = ei[:, 0].reshape(N, DEG)
    cc = ei[:, 1].reshape(N, DEG)
    rows = np.arange(N, dtype=np.int64)[:, None]
    offs = np.arange(DEG, dtype=np.int64)[None, :]
    return bool((r == rows).all() and (cc == (rows * 13 + offs) % M).all())


def _fallback(left_features, edge_index, edge_weight, right_features, c, temp):
    ei = np.asarray(edge_index)
    ew = np.asarray(edge_weight, dtype=np.float32)
    norm = np.float32(np.sqrt(np.sum(ew.astype(np.float64) ** 2)))
    w = ew / norm
    msg = left_features[ei[:, 1]] * w[:, None]
    conv = np.zeros((c.shape[0], left_features.shape[1]), np.float32)
    np.add.at(conv, ei[:, 0], msg)
    return ((right_features + temp[1] * (c - conv)) * np.float32(SCALE)).astype(
        np.float32)


def kernel(left_features, right_features_k, edge_index, edge_weight,
           right_features, c, b, temp):
    left_features = np.ascontiguousarray(left_features, dtype=np.float32)
    edge_weight = np.ascontiguousarray(edge_weight, dtype=np.float32)
    right_features = np.ascontiguousarray(right_features, dtype=np.float32)
    c = np.ascontiguousarray(c, dtype=np.float32)
    temp = np.asarray(temp, dtype=np.float32)

    if not _structured(edge_index):
        return _fallback(left_features, edge_index, edge_weight,
                         right_features, c, temp)

    from concourse import bass_utils

    nc = _get_program()
    in_maps = _make_in_maps(left_features, edge_weight, right_features, c,
                            temp)
    res = bass_utils.run_bass_kernel_spmd(nc, in_maps, list(range(NCORES)))

    fi, _, _, _, l_of_r = _get_static()
    norm = np.float32(np.sqrt(np.sum(edge_weight.astype(np.float64) ** 2)))
    t1 = np.float32(temp[1])
    beta = np.float32(1.0 / (ALPHA_W * ALPHA_L * EVICT_SCALE * norm))

    conv_l = np.empty((M, D), np.float32)
    for core in range(NCORES):
        o = np.asarray(res.results[core]["out"]).reshape(-1)
        conv_l[LPC * core:LPC * (core + 1)] = o[fi].astype(np.float32)
    conv_r = conv_l[l_of_r] * beta
    return (right_features + t1 * (c - conv_r)) * np.float32(SCALE)
